# revision 56
# baseline (speedup 1.0000x reference)
"""Biased MHSA Trainium2 kernel (8-core SPMD), v5.

Sharding: core c -> (batch b = c//2, head-group g = c%2); each core computes
attention for 4 of the 8 heads of one batch and the partial output projection
for those heads. Host sums the two head-group partials per batch and adds
bo + bv @ wo (bv folded via softmax row-sum = 1; bk dropped entirely since a
per-query constant shift cancels in softmax).

Key structure:
  - exp(S + bias) = exp(S) * exp(bias): exp(bias) precomputed on HOST, bf16.
  - x and wq/wk/wv shipped bf16 (projection matmuls bf16, fp32 PSUM).
  - One ACT exp per head-pair on a [128,1024] scores PSUM tile (double
    buffered); DVE multiplies by exp(bias) in bf16 2x-packed mode.
  - PV accumulates into 4 single-bank [65,512] PSUM tiles (bf16 V + ones
    column = softmax denominator). PE emission interleaves PV of kt-1 after
    the score MMs of kt so the PE never waits on the exp->mult chain.
  - Projections for token chunks 1-3 are interleaved into attention chunk
    0's key loop; O-proj tiles of chunk qc-1 are interleaved into chunk
    qc's key loop. Both keep the PE dense so the HAM activity monitor never
    re-throttles the PE clock to half rate.
  - Softmax normalization is pipelined across chunks with at most ONE
    deferred DVE op per key-tile (the in-order DVE queue must never delay
    the PV operands): boundary = U eviction + r-row DMAs + gather; kt3 =
    reciprocal; kt8-11 = one A^T write per kt; kt12-15 = one O-proj tile
    per kt. The last chunk instead broadcasts 1/r through the PE (ones
    matmul into the freed PSUM accumulator banks) to avoid the slow DMA
    broadcast on the critical tail.
"""

import sys

if "/opt/trn_rl_repo" not in sys.path:
    sys.path.insert(0, "/opt/trn_rl_repo")

from contextlib import ExitStack

import numpy as np
import ml_dtypes

import concourse.bass as bass
from concourse import bacc
import concourse.tile as tile
from concourse import mybir
from concourse.bass_utils import run_bass_kernel_spmd

B, N, D = 4, 2048, 512
H, DH = 8, 64
HG = 4  # heads per core
GD = HG * DH  # 256 features per core
P = 128
QC = 512  # q processed in chunks of 512
NQC = N // QC  # 4 q chunks
NKT = N // P  # 16 key tiles
KC = D // P  # 4 contraction chunks for projections
F32 = mybir.dt.float32
F32R = mybir.dt.float32r
BF16 = mybir.dt.bfloat16
BF16NP = ml_dtypes.bfloat16


def build_program():
    nc = bacc.Bacc("TRN2", target_bir_lowering=False)
    xT = nc.dram_tensor("xT", [D, N], BF16, kind="ExternalInput")
    ebT = nc.dram_tensor("ebT", [N, N], BF16, kind="ExternalInput")  # exp(bias)^T
    wq = nc.dram_tensor("wq", [D, GD], BF16, kind="ExternalInput")
    wk = nc.dram_tensor("wk", [D, GD], BF16, kind="ExternalInput")
    wv = nc.dram_tensor("wv", [D, GD], BF16, kind="ExternalInput")
    wo = nc.dram_tensor("wo", [GD, D], F32R, kind="ExternalInput")
    bq = nc.dram_tensor("bq", [GD], F32, kind="ExternalInput")
    out = nc.dram_tensor("out", [N, D], F32, kind="ExternalOutput")

    with tile.TileContext(nc) as tc, ExitStack() as ctx:
        const = ctx.enter_context(tc.tile_pool(name="const", bufs=1))
        big = ctx.enter_context(tc.tile_pool(name="big", bufs=1))
        et_pool = ctx.enter_context(tc.tile_pool(name="etp", bufs=6))
        sp_pool = ctx.enter_context(tc.tile_pool(name="spp", bufs=4))
        spb_pool = ctx.enter_context(tc.tile_pool(name="spbp", bufs=4))
        u_pool = ctx.enter_context(tc.tile_pool(name="up", bufs=2))
        r_pool = ctx.enter_context(tc.tile_pool(name="rp", bufs=2))
        o_pool = ctx.enter_context(tc.tile_pool(name="op", bufs=3))
        psum_s = ctx.enter_context(tc.tile_pool(name="psum_s", bufs=2, space="PSUM"))
        psum_u = ctx.enter_context(tc.tile_pool(name="psum_u", bufs=1, space="PSUM"))
        dram_p = ctx.enter_context(tc.tile_pool(name="dram_p", bufs=2, space="DRAM"))

        # ---- input DMAs: sync queue carries xT c0/c1 + the et stream;
        # scalar (ACT) hw queue carries weights + xT c2/c3 (ACT idle early)
        xT_s = big.tile([P, KC, N], BF16)  # x^T as [128, kc, tok]
        xT_r = xT.rearrange("(kc p) n -> p kc n", p=P)
        nc.sync.dma_start(out=xT_s[:, :, 0:QC], in_=xT_r[:, :, 0:QC])
        wk_s = const.tile([P, KC, GD], BF16)
        nc.scalar.dma_start(out=wk_s, in_=wk.rearrange("(kc p) f -> p kc f", p=P))
        nc.sync.dma_start(out=xT_s[:, :, QC : 2 * QC], in_=xT_r[:, :, QC : 2 * QC])
        wv_s = const.tile([P, KC, GD], BF16)
        nc.scalar.dma_start(out=wv_s, in_=wv.rearrange("(kc p) f -> p kc f", p=P))
        wq_s = const.tile([P, KC, GD], BF16)
        nc.scalar.dma_start(out=wq_s, in_=wq.rearrange("(kc p) f -> p kc f", p=P))
        bq_s = const.tile([P, 2], F32)
        nc.scalar.dma_start(out=bq_s, in_=bq.rearrange("(fc p) -> p fc", p=P))
        for c in range(2, NQC):
            csl = slice(c * QC, (c + 1) * QC)
            nc.scalar.dma_start(out=xT_s[:, :, csl], in_=xT_r[:, :, csl])
        # wo rows packed 2 heads per 128: wo2[p, j, :] = wo[j*128 + p, :]
        wo2_s = const.tile([P, 2, D], F32R)
        nc.scalar.dma_start(out=wo2_s, in_=wo.rearrange("(j p) d -> p j d", p=P))

        def et_dma(qc, kt):
            # exp(bias)^T tile [128 keys, 512 q], DOUBLED along the free dim
            # (one repeat-AP DMA) so each head-pair's softmax multiply is a
            # single wide DVE op in 2x-packed mode
            bt = et_pool.tile([P, 2, QC], BF16, tag="et", name="et")
            src = ebT[kt * P : (kt + 1) * P, qc * QC : (qc + 1) * QC]
            nc.sync.dma_start(
                out=bt,
                in_=bass.AP(
                    tensor=src.tensor, offset=src.offset,
                    ap=[list(src.ap[0]), [0, 2], list(src.ap[1])],
                ),
            )
            return bt

        bts = {}
        for kt in range(3):
            bts[kt] = et_dma(0, kt)

        bqs = const.tile([P, 2], F32)  # bq * 0.125 (scale folded into Q)
        nc.vector.tensor_scalar_mul(bqs, bq_s, 0.125)
        ones97 = const.tile([97, DH], F32R)
        nc.vector.memset(ones97.bitcast(F32), 1.0)

        # Q^T, K^T: [128, fc, tok]; head h lives at partitions (h%2)*64 of
        # chunk fc=h//2 (so head pair j=(2j,2j+1) occupies all of fc=j).
        qT = big.tile([P, 2, N], BF16)
        kT = big.tile([P, 2, N], BF16)
        # V natural layout, bf16, augmented ones column: vaug[128tok, h, kt, 65]
        vaug = big.tile([P, HG, NKT, DH + 1], BF16)
        nc.vector.memset(vaug[:, :, :, DH : DH + 1], 1.0)
        # A^T 2-head-packed: aT2[p, j, q]; partitions 0:64 = head 2j,
        # 64:128 = head 2j+1 (matches wo2_s packing).
        aT2 = big.tile([P, 2, N], F32R)

        def proj_kv(c):
            # K, V projections for token chunk c
            csl = slice(c * QC, (c + 1) * QC)
            for fc in range(2):
                ps = psum_s.tile([P, QC], F32, tag="s")
                for kc in range(KC):
                    nc.tensor.matmul(
                        ps,
                        wk_s[:, kc, fc * P : (fc + 1) * P],
                        xT_s[:, kc, csl],
                        start=(kc == 0),
                        stop=(kc == KC - 1),
                    )
                nc.vector.tensor_copy(kT[:, fc, csl], ps)
            for kt in range(4 * c, 4 * c + 4):
                ps = psum_s.tile([P, GD], F32, tag="s")
                for kc in range(KC):
                    nc.tensor.matmul(
                        ps,
                        xT_s[:, kc, kt * P : (kt + 1) * P],
                        wv_s[:, kc, :],
                        start=(kc == 0),
                        stop=(kc == KC - 1),
                    )
                nc.vector.tensor_copy(
                    vaug[:, :, kt, 0:DH],
                    ps.rearrange("p (h d) -> p h d", h=HG),
                )

        def proj_q(c, fc):
            # Q projection, one 128-feature block of token chunk c
            csl = slice(c * QC, (c + 1) * QC)
            ps = psum_s.tile([P, QC], F32, tag="s")
            for kc in range(KC):
                nc.tensor.matmul(
                    ps,
                    wq_s[:, kc, fc * P : (fc + 1) * P],
                    xT_s[:, kc, csl],
                    start=(kc == 0),
                    stop=(kc == KC - 1),
                )
            # (x@wq + bq) * 0.125 == psum*0.125 + bq*0.125
            nc.vector.tensor_scalar(
                qT[:, fc, csl],
                ps,
                0.125,
                bqs[:, fc : fc + 1],
                op0=mybir.AluOpType.mult,
                op1=mybir.AluOpType.add,
            )

        # -- softmax-normalization machinery, pipelined across chunks --
        def norm_begin(qc, pu, rcg=None, rcx=None):
            st = {"qc": qc, "uts": [None] * HG, "pu": pu, "rcg": rcg, "rcx": rcx}
            if rcg is None:
                st["rr"] = r_pool.tile([HG, QC], F32, tag="rr", name="rr")
            return st

        def norm_evict(st, hs):
            # evict U (frees the PSUM accumulators), push each r row
            # (denominator) straight to its reciprocal staging partition
            # via an SBUF->SBUF DMA (no DRAM roundtrip)
            for h in hs:
                ut = u_pool.tile([DH + 1, QC], F32, tag=f"ut{h}", name=f"ut{h}")
                nc.vector.tensor_copy(ut, st["pu"][h])
                st["uts"][h] = ut
                if st["rcg"] is not None:  # tail: rows 0/32/64 + rcx for h3
                    dst = (
                        st["rcg"][32 * h : 32 * h + 1, :]
                        if h < 3
                        else st["rcx"][0:1, :]
                    )
                    (nc.sync if h % 2 == 0 else nc.scalar).dma_start(
                        out=dst, in_=ut[DH : DH + 1, :]
                    )
                else:
                    nc.gpsimd.dma_start(
                        out=st["rr"][h : h + 1, :], in_=ut[DH : DH + 1, :]
                    )

        def norm_recip(st):
            rc = r_pool.tile([HG, QC], F32, tag="rc", name="rc")
            nc.vector.reciprocal_approx_fast(out=rc, in_=st["rr"])
            st["rc"] = rc

        def norm_bcast_dma(st):
            # broadcast 1/r across 64 partitions via DRAM roundtrip (hidden
            # under the next chunk's key loop; gpsimd queue is idle there)
            rd2 = dram_p.tile([HG, QC], F32, tag="rd2", name="rd2")
            nc.gpsimd.dma_start(out=rd2[:, :], in_=st["rc"])
            rb = r_pool.tile([DH, HG, QC], F32, tag="rb", name="rb")
            rap = rd2[:, :]
            nc.gpsimd.dma_start(
                out=rb,
                in_=bass.AP(
                    tensor=rap.tensor, offset=rap.offset,
                    ap=[[0, DH]] + list(rap.ap),
                ),
            )
            st["rb_ap"] = [rb[:, h, :] for h in range(HG)]

        def norm_bcast_pe(st):
            # tail path: in-place reciprocal on the staging tile (rows were
            # gathered at partitions 0/32/64/96; the rest is memset to 1.0),
            # then broadcast through the PE into the freed accumulator banks
            rcg, rcx = st["rcg"], st["rcx"]
            nc.vector.reciprocal_approx_fast(out=rcg, in_=rcg)
            nc.vector.reciprocal_approx_fast(out=rcx, in_=rcx)
            rcgr = r_pool.tile([65, QC], F32R, tag="rcgr", name="rcgr")
            nc.vector.tensor_copy(rcgr, rcg)
            rcxr = r_pool.tile([1, QC], F32R, tag="rcxr", name="rcxr")
            nc.vector.tensor_copy(rcxr, rcx)
            aps = []
            for h in range(HG):
                src = rcgr[32 * h : 32 * h + 1, :] if h < 3 else rcxr[0:1, :]
                lhs = ones97[32 * h : 32 * h + 1, :] if h < 3 else ones97[0:1, :]
                rbp = psum_u.tile([DH, QC], F32, tag=f"u{h}", name=f"rbp{h}")
                nc.tensor.matmul(rbp, lhs, src, start=True, stop=True)
                aps.append(rbp[:, :])
            st["rb_ap"] = aps

        def norm_aT2(st, h, eng=None):
            # A^T = U^T * (1/r), written 2-head-packed. Mid-kernel this
            # runs on GPSIMD (all-SBUF operands; keeps the DVE queue free
            # for the softmax multiplies); the tail reads 1/r from PSUM so
            # it must use the DVE.
            qc = st["qc"]
            qsl = slice(qc * QC, (qc + 1) * QC)
            po = (h % 2) * DH
            (eng or nc.vector).tensor_tensor(
                aT2[po : po + DH, h // 2, qsl],
                st["uts"][h][0:DH, :],
                st["rb_ap"][h],
                op=mybir.AluOpType.mult,
            )

        def oproj_pair(t1, t2):
            # Two O-proj tiles emitted together: an EVEN number of psum_s
            # allocations keeps the scores double-buffer rotation parity
            # intact, and the ACT-engine evictions ride the exp stream's
            # natural bubble instead of jamming the DVE multiply queue.
            # O[tok, 512] = sum_j aT2[:, j, tsl].T @ wo2[:, j, :]
            for n, t in enumerate((t1, t2)):
                tsl = slice(t * P, (t + 1) * P)
                po = psum_s.tile([P, D], F32, tag="s", name="po")
                for j in range(2):
                    nc.tensor.matmul(
                        po, aT2[:, j, tsl], wo2_s[:, j, :],
                        start=(j == 0), stop=(j == 1),
                    )
                ob = o_pool.tile([P, D], F32, name="ob")
                nc.scalar.activation(ob, po, mybir.ActivationFunctionType.Copy)
                (nc.sync if n == 0 else nc.gpsimd).dma_start(
                    out=out[tsl, :], in_=ob
                )

        # ---- main schedule ----
        proj_kv(0)
        proj_q(0, 0)
        proj_q(0, 1)
        norm_st = None
        rcg_tail = rcx_tail = None
        opq = []  # O-proj tiles deferred past their chunk's boundary
        for qc in range(NQC):
            qsl = slice(qc * QC, (qc + 1) * QC)
            pu = [
                psum_u.tile([DH + 1, QC], F32, tag=f"u{h}", name=f"pu{h}")
                if h < 2 else None
                for h in range(HG)
            ]
            spb_hold = [[], []]
            for kt in range(NKT):
                if kt + 2 < NKT:
                    if (qc, kt + 2) != (0, 2):  # (0,0..2) already prefetched
                        bts[kt + 2] = et_dma(qc, kt + 2)
                else:
                    if qc + 1 < NQC:
                        bts[kt + 2 - NKT] = et_dma(qc + 1, kt + 2 - NKT)
                bt = bts.pop(kt)
                # interleave remaining projections into chunk 0 (PE slack;
                # keeps HAM from re-throttling during the cold start)
                if qc == 0 and kt in (3, 7, 11):
                    c = kt // 4 + 1
                    proj_kv(c)
                    proj_q(c, 0)
                    proj_q(c, 1)
                # at kt==15 emit head-pair 1 first: the next chunk's first
                # score MM then WAR-waits on the EARLIER exp, not the later
                for j in (0, 1) if kt < NKT - 1 else (1, 0):
                    ps2 = psum_s.tile([P, 2 * QC], F32, tag="s", name="ps2")
                    for i in range(2):
                        ho = i * DH
                        nc.tensor.matmul(
                            ps2[:, i * QC : (i + 1) * QC],
                            kT[ho : ho + DH, j, kt * P : (kt + 1) * P],
                            qT[ho : ho + DH, j, qsl],
                            start=True,
                            stop=True,
                        )
                    # PE: PV for key-tile kt-2 right after this pair's score
                    # MMs — two kts of skew so DVE-queue jitter on the
                    # softmax multiplies never stalls the PE.
                    if kt > 1:
                        sprev = spb_hold[j][0]
                        for i in range(2):
                            nc.tensor.matmul(
                                pu[2 * j + i],
                                vaug[:, 2 * j + i, kt - 2, :],
                                sprev[:, i * QC : (i + 1) * QC],
                                start=(kt - 2 == 0),
                                stop=False,
                            )
                    sp = sp_pool.tile([P, 2 * QC], BF16, tag="sp", name="sp")
                    nc.scalar.activation(sp, ps2, mybir.ActivationFunctionType.Exp)
                    spb = spb_pool.tile([P, 2 * QC], BF16, tag="spb", name="spb")
                    nc.vector.tensor_tensor(
                        spb, sp, bt.rearrange("p a q -> p (a q)"),
                        op=mybir.AluOpType.mult,
                    )
                    spb_hold[j].append(spb)
                    if len(spb_hold[j]) > 2:
                        spb_hold[j].pop(0)
                # deferred work, emitted AFTER this kt's critical DVE mults
                if kt == 0:
                    if norm_st is not None:
                        norm_evict(norm_st, (2, 3))
                    pu[2] = psum_u.tile([DH + 1, QC], F32, tag="u2", name="pu2")
                    pu[3] = psum_u.tile([DH + 1, QC], F32, tag="u3", name="pu3")
                # tail staging tile for the last chunk's normalization
                if qc == NQC - 1 and kt == 10:
                    rcg_tail = r_pool.tile([65, QC], F32, tag="rcg", name="rcg")
                    nc.vector.memset(rcg_tail, 1.0)
                    rcx_tail = r_pool.tile([1, QC], F32, tag="rcx", name="rcx")
                if kt == 4 and qc >= 2:
                    # second pair of chunk qc-2 (its A^T resolved during
                    # chunk qc-1's key loop)
                    oproj_pair(4 * (qc - 2) + 2, 4 * (qc - 2) + 3)
                if norm_st is not None:
                    if kt == 3:
                        norm_recip(norm_st)
                        norm_bcast_dma(norm_st)
                    elif 10 <= kt <= 13:
                        norm_aT2(norm_st, kt - 10)
            # the first O-proj pair of qc-1 fills the boundary PE gap:
            # emitted BEFORE the final PV block, it runs as soon as the
            # kt15 exps release their PSUM buffers, while PV still waits
            # on the softmax multiplies.
            if norm_st is not None:
                oproj_pair(4 * (qc - 1), 4 * (qc - 1) + 1)
            for kk in (NKT - 2, NKT - 1):  # PV for the last two key tiles
                for j in range(2):
                    sprev = spb_hold[j][kk - (NKT - 2)]
                    for i in range(2):
                        nc.tensor.matmul(
                            pu[2 * j + i],
                            vaug[:, 2 * j + i, kk, :],
                            sprev[:, i * QC : (i + 1) * QC],
                            start=False,
                            stop=(kk == NKT - 1),
                        )
            if qc == NQC - 1:
                norm_st = norm_begin(qc, pu, rcg=rcg_tail, rcx=rcx_tail)
                norm_evict(norm_st, (0, 1, 2, 3))
            else:
                norm_st = norm_begin(qc, pu)
                norm_evict(norm_st, (0, 1))
        # tail: qc2's last pair keeps the PE busy while the last chunk's
        # PE-broadcast normalization chain resolves, then qc3's own tiles
        oproj_pair(10, 11)
        norm_bcast_pe(norm_st)
        for h in range(HG):
            norm_aT2(norm_st, h, eng=nc.vector)
        oproj_pair(12, 13)
        oproj_pair(14, 15)

    nc.compile()
    return nc


_NC = None


def _get_nc():
    global _NC
    if _NC is None:
        _NC = build_program()
    return _NC


def make_in_maps(x, attn_bias, wq, bq, wk, bk, wv, bv, wo, bo):
    x = np.asarray(x, np.float32)
    attn_bias = np.asarray(attn_bias, np.float32)
    # exp(bias)^T per batch, bf16 (shared by the 2 cores of each batch)
    ebTs = [
        np.exp(attn_bias[b, 0].T).astype(BF16NP) for b in range(B)
    ]
    xTs = [np.ascontiguousarray(x[b].T.astype(BF16NP)) for b in range(B)]
    in_maps = []
    for c in range(8):
        b, g = c // 2, c % 2
        sl = slice(g * GD, (g + 1) * GD)
        in_maps.append(
            {
                "xT": xTs[b],
                "ebT": ebTs[b],
                "wq": np.ascontiguousarray(np.asarray(wq, np.float32)[:, sl].astype(BF16NP)),
                "wk": np.ascontiguousarray(np.asarray(wk, np.float32)[:, sl].astype(BF16NP)),
                "wv": np.ascontiguousarray(np.asarray(wv, np.float32)[:, sl].astype(BF16NP)),
                "wo": np.ascontiguousarray(np.asarray(wo, np.float32)[sl, :]),
                "bq": np.ascontiguousarray(np.asarray(bq, np.float32)[sl]),
            }
        )
    return in_maps


def gather_output(results, bo, bv, wo):
    bo = np.asarray(bo, np.float32)
    row = bo + np.asarray(bv, np.float32) @ np.asarray(wo, np.float32)
    out = np.empty((B, N, D), np.float32)
    for b in range(B):
        out[b] = results[2 * b]["out"] + results[2 * b + 1]["out"] + row[None, :]
    return out


def kernel(x, attn_bias, wq, bq, wk, bk, wv, bv, wo, bo, _trace=False):
    nc = _get_nc()
    in_maps = make_in_maps(x, attn_bias, wq, bq, wk, bk, wv, bv, wo, bo)
    res = run_bass_kernel_spmd(nc, in_maps, core_ids=list(range(8)), trace=_trace)
    out = gather_output(res.results, bo, bv, wo)
    if _trace:
        kernel.last_results = res
    return out


# revision 61
# speedup vs baseline: 1.0372x; 1.0372x over previous
"""Biased MHSA Trainium2 kernel (8-core SPMD), v5.

Sharding: core c -> (batch b = c//2, head-group g = c%2); each core computes
attention for 4 of the 8 heads of one batch and the partial output projection
for those heads. Host sums the two head-group partials per batch and adds
bo + bv @ wo (bv folded via softmax row-sum = 1; bk dropped entirely since a
per-query constant shift cancels in softmax).

Key structure:
  - exp(S + bias) = exp(S) * exp(bias): exp(bias) precomputed on HOST, bf16.
  - x and wq/wk/wv shipped bf16 (projection matmuls bf16, fp32 PSUM).
  - One ACT exp per head-pair on a [128,1024] scores PSUM tile (double
    buffered); DVE multiplies by exp(bias) in bf16 2x-packed mode.
  - PV accumulates into 4 single-bank [65,512] PSUM tiles (bf16 V + ones
    column = softmax denominator). PE emission interleaves PV of kt-1 after
    the score MMs of kt so the PE never waits on the exp->mult chain.
  - Projections for token chunks 1-3 are interleaved into attention chunk
    0's key loop; O-proj tiles of chunk qc-1 are interleaved into chunk
    qc's key loop. Both keep the PE dense so the HAM activity monitor never
    re-throttles the PE clock to half rate.
  - Softmax normalization is pipelined across chunks with at most ONE
    deferred DVE op per key-tile (the in-order DVE queue must never delay
    the PV operands): boundary = U eviction + r-row DMAs + gather; kt3 =
    reciprocal; kt8-11 = one A^T write per kt; kt12-15 = one O-proj tile
    per kt. The last chunk instead broadcasts 1/r through the PE (ones
    matmul into the freed PSUM accumulator banks) to avoid the slow DMA
    broadcast on the critical tail.
"""

import sys

if "/opt/trn_rl_repo" not in sys.path:
    sys.path.insert(0, "/opt/trn_rl_repo")

from contextlib import ExitStack

import numpy as np
import ml_dtypes

import concourse.bass as bass
from concourse import bacc
import concourse.tile as tile
from concourse import mybir
from concourse.bass_utils import run_bass_kernel_spmd

B, N, D = 4, 2048, 512
H, DH = 8, 64
HG = 4  # heads per core
GD = HG * DH  # 256 features per core
P = 128
QC = 512  # q processed in chunks of 512
NQC = N // QC  # 4 q chunks
NKT = N // P  # 16 key tiles
KC = D // P  # 4 contraction chunks for projections
F32 = mybir.dt.float32
F32R = mybir.dt.float32r
BF16 = mybir.dt.bfloat16
BF16NP = ml_dtypes.bfloat16


def build_program():
    nc = bacc.Bacc("TRN2", target_bir_lowering=False)
    xT = nc.dram_tensor("xT", [D, N], BF16, kind="ExternalInput")
    ebT = nc.dram_tensor("ebT", [N, N], BF16, kind="ExternalInput")  # exp(bias)^T
    wq = nc.dram_tensor("wq", [D, GD], BF16, kind="ExternalInput")
    wk = nc.dram_tensor("wk", [D, GD], BF16, kind="ExternalInput")
    wv = nc.dram_tensor("wv", [D, GD], BF16, kind="ExternalInput")
    wo = nc.dram_tensor("wo", [GD, D], F32R, kind="ExternalInput")
    bq = nc.dram_tensor("bq", [GD], F32, kind="ExternalInput")
    out = nc.dram_tensor("out", [N, D], F32, kind="ExternalOutput")

    with tile.TileContext(nc) as tc, ExitStack() as ctx:
        const = ctx.enter_context(tc.tile_pool(name="const", bufs=1))
        big = ctx.enter_context(tc.tile_pool(name="big", bufs=1))
        et_pool = ctx.enter_context(tc.tile_pool(name="etp", bufs=6))
        sp_pool = ctx.enter_context(tc.tile_pool(name="spp", bufs=4))
        spb_pool = ctx.enter_context(tc.tile_pool(name="spbp", bufs=4))
        u_pool = ctx.enter_context(tc.tile_pool(name="up", bufs=2))
        r_pool = ctx.enter_context(tc.tile_pool(name="rp", bufs=2))
        o_pool = ctx.enter_context(tc.tile_pool(name="op", bufs=3))
        psum_s = ctx.enter_context(tc.tile_pool(name="psum_s", bufs=2, space="PSUM"))
        psum_u = ctx.enter_context(tc.tile_pool(name="psum_u", bufs=1, space="PSUM"))
        dram_p = ctx.enter_context(tc.tile_pool(name="dram_p", bufs=2, space="DRAM"))

        # ---- input DMAs: sync queue carries xT c0/c1 + the et stream;
        # scalar (ACT) hw queue carries weights + xT c2/c3 (ACT idle early)
        xT_s = big.tile([P, KC, N], BF16)  # x^T as [128, kc, tok]
        xT_r = xT.rearrange("(kc p) n -> p kc n", p=P)
        nc.sync.dma_start(out=xT_s[:, :, 0:QC], in_=xT_r[:, :, 0:QC])
        wk_s = const.tile([P, KC, GD], BF16)
        nc.scalar.dma_start(out=wk_s, in_=wk.rearrange("(kc p) f -> p kc f", p=P))
        nc.sync.dma_start(out=xT_s[:, :, QC : 2 * QC], in_=xT_r[:, :, QC : 2 * QC])
        wv_s = const.tile([P, KC, GD], BF16)
        nc.scalar.dma_start(out=wv_s, in_=wv.rearrange("(kc p) f -> p kc f", p=P))
        wq_s = const.tile([P, KC, GD], BF16)
        nc.scalar.dma_start(out=wq_s, in_=wq.rearrange("(kc p) f -> p kc f", p=P))
        bq_s = const.tile([P, 2], F32)
        nc.scalar.dma_start(out=bq_s, in_=bq.rearrange("(fc p) -> p fc", p=P))
        for c in range(2, NQC):
            csl = slice(c * QC, (c + 1) * QC)
            nc.scalar.dma_start(out=xT_s[:, :, csl], in_=xT_r[:, :, csl])
        # wo rows packed 2 heads per 128: wo2[p, j, :] = wo[j*128 + p, :]
        wo2_s = const.tile([P, 2, D], F32R)
        nc.scalar.dma_start(out=wo2_s, in_=wo.rearrange("(j p) d -> p j d", p=P))

        def et_dma(qc, kt):
            # exp(bias)^T tile [128 keys, 512 q], DOUBLED along the free dim
            # (one repeat-AP DMA) so each head-pair's softmax multiply is a
            # single wide DVE op in 2x-packed mode
            bt = et_pool.tile([P, 2, QC], BF16, tag="et", name="et")
            src = ebT[kt * P : (kt + 1) * P, qc * QC : (qc + 1) * QC]
            nc.sync.dma_start(
                out=bt,
                in_=bass.AP(
                    tensor=src.tensor, offset=src.offset,
                    ap=[list(src.ap[0]), [0, 2], list(src.ap[1])],
                ),
            )
            return bt

        bts = {}
        for kt in range(3):
            bts[kt] = et_dma(0, kt)

        bqs = const.tile([P, 2], F32)  # bq * 0.125 (scale folded into Q)
        nc.vector.tensor_scalar_mul(bqs, bq_s, 0.125)
        ones97 = const.tile([97, DH], F32R)
        nc.vector.memset(ones97.bitcast(F32), 1.0)

        # Q^T, K^T: [128, fc, tok]; head h lives at partitions (h%2)*64 of
        # chunk fc=h//2 (so head pair j=(2j,2j+1) occupies all of fc=j).
        qT = big.tile([P, 2, N], BF16)
        kT = big.tile([P, 2, N], BF16)
        # V natural layout, bf16, augmented ones column: vaug[128tok, h, kt, 65]
        vaug = big.tile([P, HG, NKT, DH + 1], BF16)
        nc.vector.memset(vaug[:, :, :, DH : DH + 1], 1.0)
        # A^T 2-head-packed: aT2[p, j, q]; partitions 0:64 = head 2j,
        # 64:128 = head 2j+1 (matches wo2_s packing).
        aT2 = big.tile([P, 2, N], F32R)

        def proj_kv(c):
            # K, V projections for token chunk c
            csl = slice(c * QC, (c + 1) * QC)
            for fc in range(2):
                ps = psum_s.tile([P, QC], F32, tag="s")
                for kc in range(KC):
                    nc.tensor.matmul(
                        ps,
                        wk_s[:, kc, fc * P : (fc + 1) * P],
                        xT_s[:, kc, csl],
                        start=(kc == 0),
                        stop=(kc == KC - 1),
                    )
                nc.vector.tensor_copy(kT[:, fc, csl], ps)
            for kt in range(4 * c, 4 * c + 4):
                ps = psum_s.tile([P, GD], F32, tag="s")
                for kc in range(KC):
                    nc.tensor.matmul(
                        ps,
                        xT_s[:, kc, kt * P : (kt + 1) * P],
                        wv_s[:, kc, :],
                        start=(kc == 0),
                        stop=(kc == KC - 1),
                    )
                nc.vector.tensor_copy(
                    vaug[:, :, kt, 0:DH],
                    ps.rearrange("p (h d) -> p h d", h=HG),
                )

        def proj_q(c, fc):
            # Q projection, one 128-feature block of token chunk c
            csl = slice(c * QC, (c + 1) * QC)
            ps = psum_s.tile([P, QC], F32, tag="s")
            for kc in range(KC):
                nc.tensor.matmul(
                    ps,
                    wq_s[:, kc, fc * P : (fc + 1) * P],
                    xT_s[:, kc, csl],
                    start=(kc == 0),
                    stop=(kc == KC - 1),
                )
            # (x@wq + bq) * 0.125 == psum*0.125 + bq*0.125
            nc.vector.tensor_scalar(
                qT[:, fc, csl],
                ps,
                0.125,
                bqs[:, fc : fc + 1],
                op0=mybir.AluOpType.mult,
                op1=mybir.AluOpType.add,
            )

        # -- softmax-normalization machinery, pipelined across chunks --
        def norm_begin(qc, pu, rcg=None, rcx=None):
            st = {"qc": qc, "uts": [None] * HG, "pu": pu, "rcg": rcg, "rcx": rcx}
            if rcg is None:
                st["rr"] = r_pool.tile([HG, QC], F32, tag="rr", name="rr")
            return st

        def norm_evict(st, hs):
            # evict U (frees the PSUM accumulators), push each r row
            # (denominator) straight to its reciprocal staging partition
            # via an SBUF->SBUF DMA (no DRAM roundtrip)
            for h in hs:
                ut = u_pool.tile([DH + 1, QC], F32, tag=f"ut{h}", name=f"ut{h}")
                nc.vector.tensor_copy(ut, st["pu"][h])
                st["uts"][h] = ut
                if st["rcg"] is not None:  # tail: rows 0/32/64 + rcx for h3
                    dst = (
                        st["rcg"][32 * h : 32 * h + 1, :]
                        if h < 3
                        else st["rcx"][0:1, :]
                    )
                    (nc.sync if h % 2 == 0 else nc.scalar).dma_start(
                        out=dst, in_=ut[DH : DH + 1, :]
                    )
                else:
                    nc.gpsimd.dma_start(
                        out=st["rr"][h : h + 1, :], in_=ut[DH : DH + 1, :]
                    )

        def norm_recip(st):
            rc = r_pool.tile([HG, QC], F32, tag="rc", name="rc")
            nc.vector.reciprocal_approx_fast(out=rc, in_=st["rr"])
            st["rc"] = rc

        def norm_bcast_dma(st):
            # broadcast 1/r across 64 partitions via DRAM roundtrip (hidden
            # under the next chunk's key loop; gpsimd queue is idle there)
            rd2 = dram_p.tile([HG, QC], F32, tag="rd2", name="rd2")
            nc.gpsimd.dma_start(out=rd2[:, :], in_=st["rc"])
            rb = r_pool.tile([DH, HG, QC], F32, tag="rb", name="rb")
            rap = rd2[:, :]
            nc.gpsimd.dma_start(
                out=rb,
                in_=bass.AP(
                    tensor=rap.tensor, offset=rap.offset,
                    ap=[[0, DH]] + list(rap.ap),
                ),
            )
            st["rb_ap"] = [rb[:, h, :] for h in range(HG)]

        def norm_bcast_pe(st):
            # tail path: in-place reciprocal on the staging tile (rows were
            # gathered at partitions 0/32/64/96; the rest is memset to 1.0),
            # then broadcast through the PE into the freed accumulator banks
            rcg, rcx = st["rcg"], st["rcx"]
            nc.vector.reciprocal_approx_fast(out=rcg, in_=rcg)
            nc.vector.reciprocal_approx_fast(out=rcx, in_=rcx)
            rcgr = r_pool.tile([65, QC], F32R, tag="rcgr", name="rcgr")
            nc.vector.tensor_copy(rcgr, rcg)
            rcxr = r_pool.tile([1, QC], F32R, tag="rcxr", name="rcxr")
            nc.vector.tensor_copy(rcxr, rcx)
            aps = []
            for h in range(HG):
                src = rcgr[32 * h : 32 * h + 1, :] if h < 3 else rcxr[0:1, :]
                lhs = ones97[32 * h : 32 * h + 1, :] if h < 3 else ones97[0:1, :]
                rbp = psum_u.tile([DH, QC], F32, tag=f"u{h}", name=f"rbp{h}")
                nc.tensor.matmul(rbp, lhs, src, start=True, stop=True)
                aps.append(rbp[:, :])
            st["rb_ap"] = aps

        def norm_aT2(st, h, eng=None):
            # A^T = U^T * (1/r), written 2-head-packed. Mid-kernel this
            # runs on GPSIMD (all-SBUF operands; keeps the DVE queue free
            # for the softmax multiplies); the tail reads 1/r from PSUM so
            # it must use the DVE.
            qc = st["qc"]
            qsl = slice(qc * QC, (qc + 1) * QC)
            po = (h % 2) * DH
            (eng or nc.vector).tensor_tensor(
                aT2[po : po + DH, h // 2, qsl],
                st["uts"][h][0:DH, :],
                st["rb_ap"][h],
                op=mybir.AluOpType.mult,
            )

        def oproj_tile(t, dma_on_scalar=False, dma_on_gpsimd=False):
            # O[tok, 512] = sum_j aT2[:, j, tsl].T @ wo2[:, j, :]
            tsl = slice(t * P, (t + 1) * P)
            po = psum_s.tile([P, D], F32, tag="s", name="po")
            for j in range(2):
                nc.tensor.matmul(
                    po, aT2[:, j, tsl], wo2_s[:, j, :],
                    start=(j == 0), stop=(j == 1),
                )
            ob = o_pool.tile([P, D], F32, name="ob")
            nc.vector.tensor_copy(ob, po)
            eng = nc.gpsimd if dma_on_gpsimd else (nc.scalar if dma_on_scalar else nc.sync)
            eng.dma_start(out=out[tsl, :], in_=ob)

        # ---- main schedule ----
        proj_kv(0)
        proj_q(0, 0)
        proj_q(0, 1)
        norm_st = None
        rcg_tail = rcx_tail = None
        opq = []  # O-proj tiles deferred past their chunk's boundary
        for qc in range(NQC):
            qsl = slice(qc * QC, (qc + 1) * QC)
            pu = [
                psum_u.tile([DH + 1, QC], F32, tag=f"u{h}", name=f"pu{h}")
                if h < 2 else None
                for h in range(HG)
            ]
            spb_hold = [[], []]
            for kt in range(NKT):
                if kt + 2 < NKT:
                    if (qc, kt + 2) != (0, 2):  # (0,0..2) already prefetched
                        bts[kt + 2] = et_dma(qc, kt + 2)
                else:
                    if qc + 1 < NQC:
                        bts[kt + 2 - NKT] = et_dma(qc + 1, kt + 2 - NKT)
                bt = bts.pop(kt)
                # interleave remaining projections into chunk 0 (PE slack;
                # keeps HAM from re-throttling during the cold start)
                if qc == 0 and kt in (3, 7, 11):
                    c = kt // 4 + 1
                    proj_kv(c)
                    proj_q(c, 0)
                    proj_q(c, 1)
                # at kt==15 emit head-pair 1 first: the next chunk's first
                # score MM then WAR-waits on the EARLIER exp, not the later
                for j in (0, 1) if kt < NKT - 1 else (1, 0):
                    ps2 = psum_s.tile([P, 2 * QC], F32, tag="s", name="ps2")
                    for i in range(2):
                        ho = i * DH
                        nc.tensor.matmul(
                            ps2[:, i * QC : (i + 1) * QC],
                            kT[ho : ho + DH, j, kt * P : (kt + 1) * P],
                            qT[ho : ho + DH, j, qsl],
                            start=True,
                            stop=True,
                        )
                    # PE: PV for the PREVIOUS kt right after this pair's
                    # score MMs — exp/mult of kt-1 are long done by now.
                    if kt > 0:
                        sprev = spb_hold[j][-1]
                        for i in range(2):
                            nc.tensor.matmul(
                                pu[2 * j + i],
                                vaug[:, 2 * j + i, kt - 1, :],
                                sprev[:, i * QC : (i + 1) * QC],
                                start=(kt - 1 == 0),
                                stop=False,
                            )
                    sp = sp_pool.tile([P, 2 * QC], BF16, tag="sp", name="sp")
                    nc.scalar.activation(sp, ps2, mybir.ActivationFunctionType.Exp)
                    spb = spb_pool.tile([P, 2 * QC], BF16, tag="spb", name="spb")
                    nc.vector.tensor_tensor(
                        spb, sp, bt.rearrange("p a q -> p (a q)"),
                        op=mybir.AluOpType.mult,
                    )
                    spb_hold[j].append(spb)
                    if len(spb_hold[j]) > 2:
                        spb_hold[j].pop(0)
                # deferred work, emitted AFTER this kt's critical DVE mults
                if kt == 0:
                    if norm_st is not None:
                        norm_evict(norm_st, (2, 3))
                    pu[2] = psum_u.tile([DH + 1, QC], F32, tag="u2", name="pu2")
                    pu[3] = psum_u.tile([DH + 1, QC], F32, tag="u3", name="pu3")
                # tail staging tile for the last chunk's normalization
                if qc == NQC - 1 and kt == 10:
                    rcg_tail = r_pool.tile([65, QC], F32, tag="rcg", name="rcg")
                    nc.vector.memset(rcg_tail, 1.0)
                    rcx_tail = r_pool.tile([1, QC], F32, tag="rcx", name="rcx")
                if kt in (2, 4) and opq:
                    oproj_tile(opq.pop(0), dma_on_gpsimd=True)
                if norm_st is not None:
                    if kt == 3:
                        norm_recip(norm_st)
                        norm_bcast_dma(norm_st)
                    elif 10 <= kt <= 13:
                        norm_aT2(norm_st, kt - 10)
                    elif kt == 14:
                        oproj_tile(4 * (qc - 1), dma_on_gpsimd=True)
            # a 2nd O-proj tile of qc-1 fills the boundary PE gap: emitted
            # BEFORE the final PV pair, it runs as soon as the earlier kt15
            # exp releases its PSUM buffer, while PV still waits on the
            # softmax multiplies. The remaining two tiles run at kts 2/4 of
            # the next chunk (or the tail for the last one).
            if norm_st is not None:
                oproj_tile(4 * (qc - 1) + 1)
                opq.extend([4 * (qc - 1) + 2, 4 * (qc - 1) + 3])
            for j in range(2):  # PV for kt = NKT-1
                sprev = spb_hold[j][-1]
                for i in range(2):
                    nc.tensor.matmul(
                        pu[2 * j + i],
                        vaug[:, 2 * j + i, NKT - 1, :],
                        sprev[:, i * QC : (i + 1) * QC],
                        start=False,
                        stop=True,
                    )
            if qc == NQC - 1:
                norm_st = norm_begin(qc, pu, rcg=rcg_tail, rcx=rcx_tail)
                norm_evict(norm_st, (0, 1, 2, 3))
            else:
                norm_st = norm_begin(qc, pu)
                norm_evict(norm_st, (0, 1))
        # tail: the deferred qc2 O-proj tiles keep the PE busy while the
        # last chunk's PE-broadcast normalization chain resolves
        for t in opq:
            oproj_tile(t, dma_on_scalar=True)
        norm_bcast_pe(norm_st)
        for h in range(HG):
            norm_aT2(norm_st, h, eng=nc.vector)
        for t in range(12, 16):
            oproj_tile(t, dma_on_scalar=(t % 2 == 1))

    nc.compile()
    return nc


_NC = None


def _get_nc():
    global _NC
    if _NC is None:
        _NC = build_program()
    return _NC


def make_in_maps(x, attn_bias, wq, bq, wk, bk, wv, bv, wo, bo):
    x = np.asarray(x, np.float32)
    attn_bias = np.asarray(attn_bias, np.float32)
    # exp(bias)^T per batch, bf16 (shared by the 2 cores of each batch)
    ebTs = [
        np.exp(attn_bias[b, 0].T).astype(BF16NP) for b in range(B)
    ]
    xTs = [np.ascontiguousarray(x[b].T.astype(BF16NP)) for b in range(B)]
    in_maps = []
    for c in range(8):
        b, g = c // 2, c % 2
        sl = slice(g * GD, (g + 1) * GD)
        in_maps.append(
            {
                "xT": xTs[b],
                "ebT": ebTs[b],
                "wq": np.ascontiguousarray(np.asarray(wq, np.float32)[:, sl].astype(BF16NP)),
                "wk": np.ascontiguousarray(np.asarray(wk, np.float32)[:, sl].astype(BF16NP)),
                "wv": np.ascontiguousarray(np.asarray(wv, np.float32)[:, sl].astype(BF16NP)),
                "wo": np.ascontiguousarray(np.asarray(wo, np.float32)[sl, :]),
                "bq": np.ascontiguousarray(np.asarray(bq, np.float32)[sl]),
            }
        )
    return in_maps


def gather_output(results, bo, bv, wo):
    bo = np.asarray(bo, np.float32)
    row = bo + np.asarray(bv, np.float32) @ np.asarray(wo, np.float32)
    out = np.empty((B, N, D), np.float32)
    for b in range(B):
        out[b] = results[2 * b]["out"] + results[2 * b + 1]["out"] + row[None, :]
    return out


def kernel(x, attn_bias, wq, bq, wk, bk, wv, bv, wo, bo, _trace=False):
    nc = _get_nc()
    in_maps = make_in_maps(x, attn_bias, wq, bq, wk, bk, wv, bv, wo, bo)
    res = run_bass_kernel_spmd(nc, in_maps, core_ids=list(range(8)), trace=_trace)
    out = gather_output(res.results, bo, bv, wo)
    if _trace:
        kernel.last_results = res
    return out


# revision 64
# speedup vs baseline: 1.0549x; 1.0170x over previous
"""Biased MHSA Trainium2 kernel (8-core SPMD), v5.

Sharding: core c -> (batch b = c//2, head-group g = c%2); each core computes
attention for 4 of the 8 heads of one batch and the partial output projection
for those heads. Host sums the two head-group partials per batch and adds
bo + bv @ wo (bv folded via softmax row-sum = 1; bk dropped entirely since a
per-query constant shift cancels in softmax).

Key structure:
  - exp(S + bias) = exp(S) * exp(bias): exp(bias) precomputed on HOST, bf16.
  - x and wq/wk/wv shipped bf16 (projection matmuls bf16, fp32 PSUM).
  - One ACT exp per head-pair on a [128,1024] scores PSUM tile (double
    buffered); DVE multiplies by exp(bias) in bf16 2x-packed mode.
  - PV accumulates into 4 single-bank [65,512] PSUM tiles (bf16 V + ones
    column = softmax denominator). PE emission interleaves PV of kt-1 after
    the score MMs of kt so the PE never waits on the exp->mult chain.
  - Projections for token chunks 1-3 are interleaved into attention chunk
    0's key loop; O-proj tiles of chunk qc-1 are interleaved into chunk
    qc's key loop. Both keep the PE dense so the HAM activity monitor never
    re-throttles the PE clock to half rate.
  - Softmax normalization is pipelined across chunks with at most ONE
    deferred DVE op per key-tile (the in-order DVE queue must never delay
    the PV operands): boundary = U eviction + r-row DMAs + gather; kt3 =
    reciprocal; kt8-11 = one A^T write per kt; kt12-15 = one O-proj tile
    per kt. The last chunk instead broadcasts 1/r through the PE (ones
    matmul into the freed PSUM accumulator banks) to avoid the slow DMA
    broadcast on the critical tail.
"""

import sys

if "/opt/trn_rl_repo" not in sys.path:
    sys.path.insert(0, "/opt/trn_rl_repo")

from contextlib import ExitStack

import numpy as np
import ml_dtypes

import concourse.bass as bass
from concourse import bacc
import concourse.tile as tile
from concourse import mybir
from concourse.bass_utils import run_bass_kernel_spmd

B, N, D = 4, 2048, 512
H, DH = 8, 64
HG = 4  # heads per core
GD = HG * DH  # 256 features per core
P = 128
QC = 512  # q processed in chunks of 512
NQC = N // QC  # 4 q chunks
NKT = N // P  # 16 key tiles
KC = D // P  # 4 contraction chunks for projections
F32 = mybir.dt.float32
F32R = mybir.dt.float32r
BF16 = mybir.dt.bfloat16
BF16NP = ml_dtypes.bfloat16


def build_program():
    nc = bacc.Bacc("TRN2", target_bir_lowering=False)
    xT = nc.dram_tensor("xT", [D, N], BF16, kind="ExternalInput")
    ebT = nc.dram_tensor("ebT", [N, N], BF16, kind="ExternalInput")  # exp(bias)^T
    wq = nc.dram_tensor("wq", [D, GD], BF16, kind="ExternalInput")
    wk = nc.dram_tensor("wk", [D, GD], BF16, kind="ExternalInput")
    wv = nc.dram_tensor("wv", [D, GD], BF16, kind="ExternalInput")
    wo = nc.dram_tensor("wo", [GD, D], F32R, kind="ExternalInput")
    bq = nc.dram_tensor("bq", [GD], F32, kind="ExternalInput")
    out = nc.dram_tensor("out", [N, D], F32, kind="ExternalOutput")

    with tile.TileContext(nc) as tc, ExitStack() as ctx:
        const = ctx.enter_context(tc.tile_pool(name="const", bufs=1))
        big = ctx.enter_context(tc.tile_pool(name="big", bufs=1))
        et_pool = ctx.enter_context(tc.tile_pool(name="etp", bufs=6))
        sp_pool = ctx.enter_context(tc.tile_pool(name="spp", bufs=4))
        spb_pool = ctx.enter_context(tc.tile_pool(name="spbp", bufs=4))
        u_pool = ctx.enter_context(tc.tile_pool(name="up", bufs=2))
        r_pool = ctx.enter_context(tc.tile_pool(name="rp", bufs=2))
        o_pool = ctx.enter_context(tc.tile_pool(name="op", bufs=3))
        psum_s = ctx.enter_context(tc.tile_pool(name="psum_s", bufs=2, space="PSUM"))
        psum_u = ctx.enter_context(tc.tile_pool(name="psum_u", bufs=1, space="PSUM"))
        dram_p = ctx.enter_context(tc.tile_pool(name="dram_p", bufs=2, space="DRAM"))

        # ---- input DMAs: sync queue carries xT c0/c1 + the et stream;
        # scalar (ACT) hw queue carries weights + xT c2/c3 (ACT idle early)
        xT_s = big.tile([P, KC, N], BF16)  # x^T as [128, kc, tok]
        xT_r = xT.rearrange("(kc p) n -> p kc n", p=P)
        nc.sync.dma_start(out=xT_s[:, :, 0:QC], in_=xT_r[:, :, 0:QC])
        wk_s = const.tile([P, KC, GD], BF16)
        nc.scalar.dma_start(out=wk_s, in_=wk.rearrange("(kc p) f -> p kc f", p=P))
        nc.sync.dma_start(out=xT_s[:, :, QC : 2 * QC], in_=xT_r[:, :, QC : 2 * QC])
        wv_s = const.tile([P, KC, GD], BF16)
        nc.scalar.dma_start(out=wv_s, in_=wv.rearrange("(kc p) f -> p kc f", p=P))
        wq_s = const.tile([P, KC, GD], BF16)
        nc.scalar.dma_start(out=wq_s, in_=wq.rearrange("(kc p) f -> p kc f", p=P))
        bq_s = const.tile([P, 2], F32)
        nc.scalar.dma_start(out=bq_s, in_=bq.rearrange("(fc p) -> p fc", p=P))
        for c in range(2, NQC):
            csl = slice(c * QC, (c + 1) * QC)
            nc.scalar.dma_start(out=xT_s[:, :, csl], in_=xT_r[:, :, csl])
        # wo rows packed 2 heads per 128: wo2[p, j, :] = wo[j*128 + p, :]
        wo2_s = const.tile([P, 2, D], F32R)
        nc.scalar.dma_start(out=wo2_s, in_=wo.rearrange("(j p) d -> p j d", p=P))

        def et_dma(qc, kt):
            # exp(bias)^T tile [128 keys, 512 q], DOUBLED along the free dim
            # (one repeat-AP DMA) so each head-pair's softmax multiply is a
            # single wide DVE op in 2x-packed mode
            bt = et_pool.tile([P, 2, QC], BF16, tag="et", name="et")
            src = ebT[kt * P : (kt + 1) * P, qc * QC : (qc + 1) * QC]
            nc.sync.dma_start(
                out=bt,
                in_=bass.AP(
                    tensor=src.tensor, offset=src.offset,
                    ap=[list(src.ap[0]), [0, 2], list(src.ap[1])],
                ),
            )
            return bt

        bts = {}
        for kt in range(3):
            bts[kt] = et_dma(0, kt)
        # (the per-kt loop prefetches 3 ahead from here on)

        bqs = const.tile([P, 2], F32)  # bq * 0.125 (scale folded into Q)
        nc.vector.tensor_scalar_mul(bqs, bq_s, 0.125)
        ones97 = const.tile([97, DH], F32R)
        nc.vector.memset(ones97.bitcast(F32), 1.0)

        # Q^T, K^T: [128, fc, tok]; head h lives at partitions (h%2)*64 of
        # chunk fc=h//2 (so head pair j=(2j,2j+1) occupies all of fc=j).
        qT = big.tile([P, 2, N], BF16)
        kT = big.tile([P, 2, N], BF16)
        # V natural layout, bf16, augmented ones column: vaug[128tok, h, kt, 65]
        vaug = big.tile([P, HG, NKT, DH + 1], BF16)
        nc.vector.memset(vaug[:, :, :, DH : DH + 1], 1.0)
        # A^T 2-head-packed: aT2[p, j, q]; partitions 0:64 = head 2j,
        # 64:128 = head 2j+1 (matches wo2_s packing).
        aT2 = big.tile([P, 2, N], F32R)

        def proj_kv(c):
            # K, V projections for token chunk c
            csl = slice(c * QC, (c + 1) * QC)
            for fc in range(2):
                ps = psum_s.tile([P, QC], F32, tag="s")
                for kc in range(KC):
                    nc.tensor.matmul(
                        ps,
                        wk_s[:, kc, fc * P : (fc + 1) * P],
                        xT_s[:, kc, csl],
                        start=(kc == 0),
                        stop=(kc == KC - 1),
                    )
                nc.vector.tensor_copy(kT[:, fc, csl], ps)
            for kt in range(4 * c, 4 * c + 4):
                ps = psum_s.tile([P, GD], F32, tag="s")
                for kc in range(KC):
                    nc.tensor.matmul(
                        ps,
                        xT_s[:, kc, kt * P : (kt + 1) * P],
                        wv_s[:, kc, :],
                        start=(kc == 0),
                        stop=(kc == KC - 1),
                    )
                nc.vector.tensor_copy(
                    vaug[:, :, kt, 0:DH],
                    ps.rearrange("p (h d) -> p h d", h=HG),
                )

        def proj_q(c, fc):
            # Q projection, one 128-feature block of token chunk c
            csl = slice(c * QC, (c + 1) * QC)
            ps = psum_s.tile([P, QC], F32, tag="s")
            for kc in range(KC):
                nc.tensor.matmul(
                    ps,
                    wq_s[:, kc, fc * P : (fc + 1) * P],
                    xT_s[:, kc, csl],
                    start=(kc == 0),
                    stop=(kc == KC - 1),
                )
            # (x@wq + bq) * 0.125 == psum*0.125 + bq*0.125
            nc.vector.tensor_scalar(
                qT[:, fc, csl],
                ps,
                0.125,
                bqs[:, fc : fc + 1],
                op0=mybir.AluOpType.mult,
                op1=mybir.AluOpType.add,
            )

        # -- softmax-normalization machinery, pipelined across chunks --
        def norm_begin(qc, pu, rcg=None, rcx=None):
            st = {"qc": qc, "uts": [None] * HG, "pu": pu, "rcg": rcg, "rcx": rcx}
            if rcg is None:
                st["rr"] = r_pool.tile([HG, QC], F32, tag="rr", name="rr")
            return st

        def norm_evict(st, hs):
            # evict U (frees the PSUM accumulators), push each r row
            # (denominator) straight to its reciprocal staging partition
            # via an SBUF->SBUF DMA (no DRAM roundtrip)
            for h in hs:
                ut = u_pool.tile([DH + 1, QC], F32, tag=f"ut{h}", name=f"ut{h}")
                nc.vector.tensor_copy(ut, st["pu"][h])
                st["uts"][h] = ut
                if st["rcg"] is not None:  # tail: rows 0/32/64 + rcx for h3
                    dst = (
                        st["rcg"][32 * h : 32 * h + 1, :]
                        if h < 3
                        else st["rcx"][0:1, :]
                    )
                    (nc.sync if h % 2 == 0 else nc.scalar).dma_start(
                        out=dst, in_=ut[DH : DH + 1, :]
                    )
                else:
                    nc.gpsimd.dma_start(
                        out=st["rr"][h : h + 1, :], in_=ut[DH : DH + 1, :]
                    )

        def norm_recip(st):
            rc = r_pool.tile([HG, QC], F32, tag="rc", name="rc")
            nc.vector.reciprocal_approx_fast(out=rc, in_=st["rr"])
            st["rc"] = rc

        def norm_bcast_dma(st):
            # broadcast 1/r across 64 partitions via DRAM roundtrip (hidden
            # under the next chunk's key loop; gpsimd queue is idle there)
            rd2 = dram_p.tile([HG, QC], F32, tag="rd2", name="rd2")
            nc.gpsimd.dma_start(out=rd2[:, :], in_=st["rc"])
            rb = r_pool.tile([DH, HG, QC], F32, tag="rb", name="rb")
            rap = rd2[:, :]
            nc.gpsimd.dma_start(
                out=rb,
                in_=bass.AP(
                    tensor=rap.tensor, offset=rap.offset,
                    ap=[[0, DH]] + list(rap.ap),
                ),
            )
            st["rb_ap"] = [rb[:, h, :] for h in range(HG)]

        def norm_bcast_pe(st):
            # tail path: in-place reciprocal on the staging tile (rows were
            # gathered at partitions 0/32/64/96; the rest is memset to 1.0),
            # then broadcast through the PE into the freed accumulator banks
            rcg, rcx = st["rcg"], st["rcx"]
            nc.vector.reciprocal_approx_fast(out=rcg, in_=rcg)
            nc.vector.reciprocal_approx_fast(out=rcx, in_=rcx)
            rcgr = r_pool.tile([65, QC], F32R, tag="rcgr", name="rcgr")
            nc.vector.tensor_copy(rcgr, rcg)
            rcxr = r_pool.tile([1, QC], F32R, tag="rcxr", name="rcxr")
            nc.vector.tensor_copy(rcxr, rcx)
            aps = []
            for h in range(HG):
                src = rcgr[32 * h : 32 * h + 1, :] if h < 3 else rcxr[0:1, :]
                lhs = ones97[32 * h : 32 * h + 1, :] if h < 3 else ones97[0:1, :]
                rbp = psum_u.tile([DH, QC], F32, tag=f"u{h}", name=f"rbp{h}")
                nc.tensor.matmul(rbp, lhs, src, start=True, stop=True)
                aps.append(rbp[:, :])
            st["rb_ap"] = aps

        def norm_aT2(st, h, eng=None):
            # A^T = U^T * (1/r), written 2-head-packed. Mid-kernel this
            # runs on GPSIMD (all-SBUF operands; keeps the DVE queue free
            # for the softmax multiplies); the tail reads 1/r from PSUM so
            # it must use the DVE.
            qc = st["qc"]
            qsl = slice(qc * QC, (qc + 1) * QC)
            po = (h % 2) * DH
            (eng or nc.vector).tensor_tensor(
                aT2[po : po + DH, h // 2, qsl],
                st["uts"][h][0:DH, :],
                st["rb_ap"][h],
                op=mybir.AluOpType.mult,
            )

        def oproj_tile(t, dma_on_scalar=False, dma_on_gpsimd=False):
            # O[tok, 512] = sum_j aT2[:, j, tsl].T @ wo2[:, j, :]
            tsl = slice(t * P, (t + 1) * P)
            po = psum_s.tile([P, D], F32, tag="s", name="po")
            for j in range(2):
                nc.tensor.matmul(
                    po, aT2[:, j, tsl], wo2_s[:, j, :],
                    start=(j == 0), stop=(j == 1),
                )
            ob = o_pool.tile([P, D], F32, name="ob")
            nc.vector.tensor_copy(ob, po)
            eng = nc.gpsimd if dma_on_gpsimd else (nc.scalar if dma_on_scalar else nc.sync)
            eng.dma_start(out=out[tsl, :], in_=ob)

        # ---- main schedule ----
        proj_kv(0)
        proj_q(0, 0)
        proj_q(0, 1)
        norm_st = None
        rcg_tail = rcx_tail = None
        opq = []  # O-proj tiles deferred past their chunk's boundary
        for qc in range(NQC):
            qsl = slice(qc * QC, (qc + 1) * QC)
            pu = [
                psum_u.tile([DH + 1, QC], F32, tag=f"u{h}", name=f"pu{h}")
                if h < 2 else None
                for h in range(HG)
            ]
            spb_hold = [[], []]
            for kt in range(NKT):
                if kt + 3 < NKT:
                    if (qc, kt + 3) > (0, 2):  # (0,0..2) already prefetched
                        bts[kt + 3] = et_dma(qc, kt + 3)
                else:
                    if qc + 1 < NQC:
                        bts[kt + 3 - NKT] = et_dma(qc + 1, kt + 3 - NKT)
                bt = bts.pop(kt)
                # interleave remaining projections into chunk 0 (PE slack;
                # keeps HAM from re-throttling during the cold start)
                if qc == 0 and kt in (3, 7, 11):
                    c = kt // 4 + 1
                    proj_kv(c)
                    proj_q(c, 0)
                    proj_q(c, 1)
                # at kt==15 emit head-pair 1 first: the next chunk's first
                # score MM then WAR-waits on the EARLIER exp, not the later
                for j in (0, 1) if kt < NKT - 1 else (1, 0):
                    ps2 = psum_s.tile([P, 2 * QC], F32, tag="s", name="ps2")
                    for i in range(2):
                        ho = i * DH
                        nc.tensor.matmul(
                            ps2[:, i * QC : (i + 1) * QC],
                            kT[ho : ho + DH, j, kt * P : (kt + 1) * P],
                            qT[ho : ho + DH, j, qsl],
                            start=True,
                            stop=True,
                        )
                    # PE: PV for the PREVIOUS kt right after this pair's
                    # score MMs — exp/mult of kt-1 are long done by now.
                    if kt > 0:
                        sprev = spb_hold[j][-1]
                        for i in range(2):
                            nc.tensor.matmul(
                                pu[2 * j + i],
                                vaug[:, 2 * j + i, kt - 1, :],
                                sprev[:, i * QC : (i + 1) * QC],
                                start=(kt - 1 == 0),
                                stop=False,
                            )
                    sp = sp_pool.tile([P, 2 * QC], BF16, tag="sp", name="sp")
                    nc.scalar.activation(sp, ps2, mybir.ActivationFunctionType.Exp)
                    spb = spb_pool.tile([P, 2 * QC], BF16, tag="spb", name="spb")
                    nc.vector.tensor_tensor(
                        spb, sp, bt.rearrange("p a q -> p (a q)"),
                        op=mybir.AluOpType.mult,
                    )
                    spb_hold[j].append(spb)
                    if len(spb_hold[j]) > 2:
                        spb_hold[j].pop(0)
                # deferred work, emitted AFTER this kt's critical DVE mults
                if kt == 0:
                    if norm_st is not None:
                        norm_evict(norm_st, (2, 3))
                    pu[2] = psum_u.tile([DH + 1, QC], F32, tag="u2", name="pu2")
                    pu[3] = psum_u.tile([DH + 1, QC], F32, tag="u3", name="pu3")
                # tail staging tile for the last chunk's normalization
                if qc == NQC - 1 and kt == 10:
                    rcg_tail = r_pool.tile([65, QC], F32, tag="rcg", name="rcg")
                    nc.vector.memset(rcg_tail, 1.0)
                    rcx_tail = r_pool.tile([1, QC], F32, tag="rcx", name="rcx")
                if kt in (2, 4) and opq:
                    oproj_tile(opq.pop(0), dma_on_gpsimd=True)
                if norm_st is not None:
                    if kt == 3:
                        norm_recip(norm_st)
                        norm_bcast_dma(norm_st)
                    elif 10 <= kt <= 13:
                        norm_aT2(norm_st, kt - 10)
                    elif kt == 14:
                        oproj_tile(4 * (qc - 1), dma_on_gpsimd=True)
            # a 2nd O-proj tile of qc-1 fills the boundary PE gap: emitted
            # BEFORE the final PV pair, it runs as soon as the earlier kt15
            # exp releases its PSUM buffer, while PV still waits on the
            # softmax multiplies. The remaining two tiles run at kts 2/4 of
            # the next chunk (or the tail for the last one).
            if norm_st is not None:
                oproj_tile(4 * (qc - 1) + 1)
                opq.extend([4 * (qc - 1) + 2, 4 * (qc - 1) + 3])
            for j in range(2):  # PV for kt = NKT-1
                sprev = spb_hold[j][-1]
                for i in range(2):
                    nc.tensor.matmul(
                        pu[2 * j + i],
                        vaug[:, 2 * j + i, NKT - 1, :],
                        sprev[:, i * QC : (i + 1) * QC],
                        start=False,
                        stop=True,
                    )
            if qc == NQC - 1:
                norm_st = norm_begin(qc, pu, rcg=rcg_tail, rcx=rcx_tail)
                norm_evict(norm_st, (0, 1, 2, 3))
            else:
                norm_st = norm_begin(qc, pu)
                norm_evict(norm_st, (0, 1))
        # tail: the deferred qc2 O-proj tiles keep the PE busy while the
        # last chunk's PE-broadcast normalization chain resolves
        for t in opq:
            oproj_tile(t, dma_on_scalar=True)
        norm_bcast_pe(norm_st)
        for h in range(HG):
            norm_aT2(norm_st, h, eng=nc.vector)
        for t in range(12, 16):
            # split the final output DMAs across both hw queues so the
            # end-of-kernel drain isn't gated on one queue's backlog
            tsl = slice(t * P, (t + 1) * P)
            po = psum_s.tile([P, D], F32, tag="s", name="po")
            for j in range(2):
                nc.tensor.matmul(
                    po, aT2[:, j, tsl], wo2_s[:, j, :],
                    start=(j == 0), stop=(j == 1),
                )
            ob = o_pool.tile([P, D], F32, name="ob")
            (nc.vector.tensor_copy if t % 2 == 0 else (
                lambda o, i: nc.scalar.activation(
                    o, i, mybir.ActivationFunctionType.Copy)
            ))(ob, po)
            nc.sync.dma_start(out=out[tsl, 0 : D // 2], in_=ob[:, 0 : D // 2])
            nc.scalar.dma_start(out=out[tsl, D // 2 : D], in_=ob[:, D // 2 : D])

    nc.compile()
    return nc


_NC = None


def _get_nc():
    global _NC
    if _NC is None:
        _NC = build_program()
    return _NC


def make_in_maps(x, attn_bias, wq, bq, wk, bk, wv, bv, wo, bo):
    x = np.asarray(x, np.float32)
    attn_bias = np.asarray(attn_bias, np.float32)
    # exp(bias)^T per batch, bf16 (shared by the 2 cores of each batch)
    ebTs = [
        np.exp(attn_bias[b, 0].T).astype(BF16NP) for b in range(B)
    ]
    xTs = [np.ascontiguousarray(x[b].T.astype(BF16NP)) for b in range(B)]
    in_maps = []
    for c in range(8):
        b, g = c // 2, c % 2
        sl = slice(g * GD, (g + 1) * GD)
        in_maps.append(
            {
                "xT": xTs[b],
                "ebT": ebTs[b],
                "wq": np.ascontiguousarray(np.asarray(wq, np.float32)[:, sl].astype(BF16NP)),
                "wk": np.ascontiguousarray(np.asarray(wk, np.float32)[:, sl].astype(BF16NP)),
                "wv": np.ascontiguousarray(np.asarray(wv, np.float32)[:, sl].astype(BF16NP)),
                "wo": np.ascontiguousarray(np.asarray(wo, np.float32)[sl, :]),
                "bq": np.ascontiguousarray(np.asarray(bq, np.float32)[sl]),
            }
        )
    return in_maps


def gather_output(results, bo, bv, wo):
    bo = np.asarray(bo, np.float32)
    row = bo + np.asarray(bv, np.float32) @ np.asarray(wo, np.float32)
    out = np.empty((B, N, D), np.float32)
    for b in range(B):
        out[b] = results[2 * b]["out"] + results[2 * b + 1]["out"] + row[None, :]
    return out


def kernel(x, attn_bias, wq, bq, wk, bk, wv, bv, wo, bo, _trace=False):
    nc = _get_nc()
    in_maps = make_in_maps(x, attn_bias, wq, bq, wk, bk, wv, bv, wo, bo)
    res = run_bass_kernel_spmd(nc, in_maps, core_ids=list(range(8)), trace=_trace)
    out = gather_output(res.results, bo, bv, wo)
    if _trace:
        kernel.last_results = res
    return out
